# revision 3
# baseline (speedup 1.0000x reference)
"""Trainium2 Bass kernel v2 for nn_BasicEncoder (embedding-lookup encoder).

Same math/layout as kernel.py (gather-and-sum of embedding columns with the
GpSimd ap_gather, block-diagonal tail matmuls), with the table load reworked:

v1 loaded the [16, V] f32 table from HBM 8x (one DMA per 16-partition
replica group) = 16.4MB of DMA per iteration, ~144us serial before the
first gather.  v2 ships the table once as bf16 [64, V/4] (1MB), then
replicates it to the [128, V] f32 gather table ON-DEVICE with the idle
TensorEngine: a 16->128 broadcast matmul per vocab chunk (stationary is a
0/1 replication matrix), PSUM->SBUF copies alternating between the idle
Activation engine and the DVE.  The staging load and PE matmuls overlap the
previous iteration's gathers (staging pool bufs=2); only the ~16us copy
phase is serial with the gathers.
"""

import numpy as np

COPYW = 1024  # PSUM->SBUF copy tile width (2 PSUM banks), tuned on HW

B, S, V, H, O = 2048, 512, 32128, 16, 16
NCORES = 8
RPC = B // NCORES  # rows per core (256)
P = 128
G = 8  # partition groups (= Q7 cores)
SLOTS = RPC // G  # rows per group (32)
RPG = 4  # rows gathered per group per ap_gather instruction
NINST = SLOTS // RPG  # ap_gather instructions per core (8)
NI = RPG * S  # indices per group per instruction (2048)
NIW = NI // 16  # wrapped idx columns per instruction (128)
VQ = V // 4  # staging columns per quarter (8032)

_CACHE = {}


def _build_nc(repeat=1):
    import contextlib

    import concourse.bacc as bacc
    import concourse.bass as bass
    import concourse.mybir as mybir
    import concourse.tile as tile

    f32 = mybir.dt.float32
    bf16 = mybir.dt.bfloat16
    i16 = mybir.dt.int16
    nc = bacc.Bacc(None, target_bir_lowering=False)

    tblq_d = nc.dram_tensor("tblq", [P, VQ], bf16, kind="ExternalInput")
    rep_d = nc.dram_tensor("rep", [P, P], bf16, kind="ExternalInput")
    xi_d = nc.dram_tensor("xi16", [P, NINST * NIW], i16, kind="ExternalInput")
    b1_d = nc.dram_tensor("b1rep", [P, 1], f32, kind="ExternalInput")
    wm_d = nc.dram_tensor("wmbd", [P, P], f32, kind="ExternalInput")
    wl_d = nc.dram_tensor("wlbd", [P, P], f32, kind="ExternalInput")
    bm_d = nc.dram_tensor("bmrep", [P, 1], f32, kind="ExternalInput")
    bl_d = nc.dram_tensor("blrep", [P, 1], f32, kind="ExternalInput")
    out_d = nc.dram_tensor("out", [P, 2 * SLOTS], f32, kind="ExternalOutput")

    # vocab copy-tiles within one quarter: 7x1024 + 864; each tile is
    # filled by two matmuls of <=512 (PSUM bank limit) and drained by one
    # wide PSUM->SBUF copy.
    tiles = [(u, min(COPYW, VQ - u)) for u in range(0, VQ, COPYW)]

    with tile.TileContext(nc) as tc:
        with (
            tc.tile_pool(name="sb", bufs=1) as pool,
            tc.tile_pool(name="stg", bufs=2) as spool,
            tc.tile_pool(name="gth", bufs=3) as gpool,
            tc.tile_pool(name="ps", bufs=1, space=bass.MemorySpace.PSUM) as pspool,
            tc.tile_pool(name="psr", bufs=3, space=bass.MemorySpace.PSUM) as prpool,
            tc.For_i(0, repeat, 1) if repeat > 1 else contextlib.nullcontext(),
        ):
            rep_sb = pool.tile([P, P], bf16)
            nc.sync.dma_start(rep_sb[:], rep_d[:])
            tblq_sb = spool.tile([P, VQ], bf16)
            nc.sync.dma_start(tblq_sb[:], tblq_d[:])
            xi_sb = spool.tile([P, NINST * NIW], i16)
            nc.sync.dma_start(xi_sb[:], xi_d[:])
            b1_sb = pool.tile([P, 1], f32)
            nc.sync.dma_start(b1_sb[:], b1_d[:])
            wm_sb = pool.tile([P, P], f32)
            nc.sync.dma_start(wm_sb[:], wm_d[:])
            wl_sb = pool.tile([P, P], f32)
            nc.sync.dma_start(wl_sb[:], wl_d[:])
            bm_sb = pool.tile([P, 1], f32)
            nc.sync.dma_start(bm_sb[:], bm_d[:])
            bl_sb = pool.tile([P, 1], f32)
            nc.sync.dma_start(bl_sb[:], bl_d[:])

            # table replication: bf16 [128, VQ] staging -> f32 [128, V] table
            tbl_sb = pool.tile([P, V], f32)
            ci = 0
            for a in range(4):
                for u0, w in tiles:
                    psr = prpool.tile([P, COPYW], f32)
                    for v0 in range(0, w, 512):
                        vw = min(512, w - v0)
                        nc.tensor.matmul(
                            psr[:, v0 : v0 + vw],
                            rep_sb[32 * a : 32 * (a + 1), :],
                            tblq_sb[32 * a : 32 * (a + 1), u0 + v0 : u0 + v0 + vw],
                            tile_position=(32 * a, 0),
                        )
                    dst = tbl_sb[:, a * VQ + u0 : a * VQ + u0 + w]
                    if ci % 2 == 0:
                        nc.scalar.activation(
                            out=dst,
                            in_=psr[:, :w],
                            func=mybir.ActivationFunctionType.Copy,
                        )
                    else:
                        nc.vector.tensor_scalar_mul(dst, psr[:, :w], 1.0)
                    ci += 1

            hall = pool.tile([P, SLOTS], f32)
            for k in range(NINST):
                gath = gpool.tile([P, NI], f32)
                nc.gpsimd.ap_gather(
                    out_ap=gath[:],
                    in_ap=tbl_sb[:],
                    idxs_ap=xi_sb[:, k * NIW : (k + 1) * NIW],
                    channels=P,
                    num_elems=V,
                    d=1,
                    num_idxs=NI,
                )
                nc.vector.tensor_reduce(
                    out=hall[:, k * RPG : (k + 1) * RPG],
                    in_=gath[:].rearrange("p (r s) -> p r s", s=S),
                    axis=mybir.AxisListType.X,
                    op=mybir.AluOpType.add,
                )

            hr = pool.tile([P, SLOTS], f32)
            nc.scalar.activation(
                out=hr[:],
                in_=hall[:],
                func=mybir.ActivationFunctionType.Relu,
                bias=b1_sb[:],
            )

            om_ps = pspool.tile([P, SLOTS], f32)
            nc.tensor.matmul(om_ps[:], wm_sb[:], hr[:])
            ol_ps = pspool.tile([P, SLOTS], f32)
            nc.tensor.matmul(ol_ps[:], wl_sb[:], hr[:])

            o_sb = pool.tile([P, 2 * SLOTS], f32)
            nc.vector.tensor_scalar(
                out=o_sb[:, :SLOTS],
                in0=om_ps[:],
                scalar1=bm_sb[:],
                scalar2=None,
                op0=mybir.AluOpType.add,
            )
            nc.vector.tensor_scalar(
                out=o_sb[:, SLOTS:],
                in0=ol_ps[:],
                scalar1=bl_sb[:],
                scalar2=None,
                op0=mybir.AluOpType.add,
            )
            nc.sync.dma_start(out_d[:], o_sb[:])

    nc.compile()
    return nc


def _get_nc(repeat=1):
    key = ("nc", repeat)
    if key not in _CACHE:
        _CACHE[key] = _build_nc(repeat)
    return _CACHE[key]


def _prep_inputs(x, enc1_w, enc1_b, mean_w, mean_b, logvar_w, logvar_b):
    import ml_dtypes

    x = np.asarray(x)
    assert x.shape == (B, S)
    # row r of core c = global row c*RPC + r; within a core, row r is
    # handled by group g = r % G at slot n = r // G; instruction k covers
    # slots k*RPG .. k*RPG+RPG-1.
    xs = x.astype(np.int16).reshape(NCORES, SLOTS, G, S)  # [c, n, g, s]
    stream = xs.transpose(0, 2, 1, 3).reshape(NCORES, G, NINST, NI)  # [c,g,k,i]
    wrapped = stream.reshape(NCORES, G, NINST, NIW, 16).transpose(0, 1, 4, 2, 3)
    xi16 = np.ascontiguousarray(wrapped.reshape(NCORES, G * 16, NINST * NIW))

    w = np.asarray(enc1_w, dtype=np.float32)  # [H, V]
    # tblq[32a+h, u] = w[h, a*VQ+u] for h<16; rows 32a+16..32a+31 unused
    tblq = np.zeros((P, VQ), dtype=ml_dtypes.bfloat16)
    wq = w.reshape(H, 4, VQ)
    for a in range(4):
        tblq[32 * a : 32 * a + H, :] = wq[:, a, :].astype(ml_dtypes.bfloat16)

    rep = np.zeros((P, P), dtype=ml_dtypes.bfloat16)
    for a in range(4):
        for m in range(P):
            rep[32 * a + (m % H), m] = 1.0

    pidx = np.arange(P) % H
    b1rep = np.asarray(enc1_b, dtype=np.float32)[pidx][:, None].copy()
    bmrep = np.asarray(mean_b, dtype=np.float32)[pidx][:, None].copy()
    blrep = np.asarray(logvar_b, dtype=np.float32)[pidx][:, None].copy()

    def blockdiag(wmat):
        wmat = np.asarray(wmat, dtype=np.float32)  # [O, H]
        bd = np.zeros((P, P), dtype=np.float32)
        for g in range(G):
            bd[g * H : (g + 1) * H, g * O : (g + 1) * O] = wmat.T
        return bd

    wmbd = blockdiag(mean_w)
    wlbd = blockdiag(logvar_w)
    return [
        {
            "tblq": tblq,
            "rep": rep,
            "xi16": xi16[c],
            "b1rep": b1rep,
            "wmbd": wmbd,
            "wlbd": wlbd,
            "bmrep": bmrep,
            "blrep": blrep,
        }
        for c in range(NCORES)
    ]


def _unscramble(out_core):
    # out_core [128, 2*SLOTS]: partition 16g+o, col n -> row n*G+g
    o = out_core.reshape(G, O, 2, SLOTS)  # [g, o, {m,l}, n]
    o = o.transpose(2, 3, 0, 1)  # [{m,l}, n, g, o]
    return o.reshape(2, SLOTS * G, O)  # rows r = n*G + g


def _run(in_maps, trace=False, repeat=1):
    from concourse.bass_utils import run_bass_kernel_spmd

    nc = _get_nc(repeat)
    core_ids = list(range(NCORES))
    res = run_bass_kernel_spmd(nc, in_maps, core_ids, trace=trace)
    mean = np.empty((B, O), dtype=np.float32)
    logvar = np.empty((B, O), dtype=np.float32)
    for c in core_ids:
        ml = _unscramble(res.results[c]["out"])
        mean[c * RPC : (c + 1) * RPC] = ml[0]
        logvar[c * RPC : (c + 1) * RPC] = ml[1]
    return mean, logvar, res


def kernel(x, enc1_w, enc1_b, mean_w, mean_b, logvar_w, logvar_b):
    in_maps = _prep_inputs(x, enc1_w, enc1_b, mean_w, mean_b, logvar_w, logvar_b)
    mean, logvar, _ = _run(in_maps, trace=False)
    return mean, logvar


# revision 4
# speedup vs baseline: 1.1784x; 1.1784x over previous
"""Trainium2 Bass kernel v2 for nn_BasicEncoder (embedding-lookup encoder).

Same math/layout as kernel.py (gather-and-sum of embedding columns with the
GpSimd ap_gather, block-diagonal tail matmuls), with the table load reworked:

v1 loaded the [16, V] f32 table from HBM 8x (one DMA per 16-partition
replica group) = 16.4MB of DMA per iteration, ~144us serial before the
first gather.  v2 ships the table once as bf16 [64, V/4] (1MB), then
replicates it to the [128, V] f32 gather table ON-DEVICE with the idle
TensorEngine: a 16->128 broadcast matmul per vocab chunk (stationary is a
0/1 replication matrix), PSUM->SBUF copies alternating between the idle
Activation engine and the DVE.  The staging load and PE matmuls overlap the
previous iteration's gathers (staging pool bufs=2); only the ~16us copy
phase is serial with the gathers.
"""

import numpy as np

COPYW = 1024  # PSUM->SBUF copy tile width (2 PSUM banks), tuned on HW

B, S, V, H, O = 2048, 512, 32128, 16, 16
NCORES = 8
RPC = B // NCORES  # rows per core (256)
P = 128
G = 8  # partition groups (= Q7 cores)
SLOTS = RPC // G  # rows per group (32)
RPG = 4  # rows gathered per group per ap_gather instruction
NINST = SLOTS // RPG  # ap_gather instructions per core (8)
NI = RPG * S  # indices per group per instruction (2048)
NIW = NI // 16  # wrapped idx columns per instruction (128)
VQ = V // 4  # staging columns per quarter (8032)

_CACHE = {}


def _build_nc(repeat=1):
    import contextlib

    import concourse.bacc as bacc
    import concourse.bass as bass
    import concourse.mybir as mybir
    import concourse.tile as tile

    f32 = mybir.dt.float32
    bf16 = mybir.dt.bfloat16
    i16 = mybir.dt.int16
    nc = bacc.Bacc(None, target_bir_lowering=False)

    tblq_d = nc.dram_tensor("tblq", [P, VQ], bf16, kind="ExternalInput")
    rep_d = nc.dram_tensor("rep", [P, P], bf16, kind="ExternalInput")
    xi_d = nc.dram_tensor("xi16", [P, NINST * NIW], i16, kind="ExternalInput")
    b1_d = nc.dram_tensor("b1rep", [P, 1], f32, kind="ExternalInput")
    wm_d = nc.dram_tensor("wmbd", [P, P], f32, kind="ExternalInput")
    wl_d = nc.dram_tensor("wlbd", [P, P], f32, kind="ExternalInput")
    bm_d = nc.dram_tensor("bmrep", [P, 1], f32, kind="ExternalInput")
    bl_d = nc.dram_tensor("blrep", [P, 1], f32, kind="ExternalInput")
    out_d = nc.dram_tensor("out", [P, 2 * SLOTS], f32, kind="ExternalOutput")

    # vocab copy-tiles within one quarter: 7x1024 + 864; each tile is
    # filled by two matmuls of <=512 (PSUM bank limit) and drained by one
    # wide PSUM->SBUF copy.
    tiles = [(u, min(COPYW, VQ - u)) for u in range(0, VQ, COPYW)]

    with tile.TileContext(nc) as tc:
        with (
            tc.tile_pool(name="sb", bufs=1) as pool,
            tc.tile_pool(name="stg", bufs=2) as spool,
            tc.tile_pool(name="gth", bufs=3) as gpool,
            tc.tile_pool(name="ps", bufs=1, space=bass.MemorySpace.PSUM) as pspool,
            tc.tile_pool(name="psr", bufs=2, space=bass.MemorySpace.PSUM) as prpool,
            tc.For_i(0, repeat, 1) if repeat > 1 else contextlib.nullcontext(),
        ):
            rep_sb = pool.tile([P, P], bf16)
            nc.sync.dma_start(rep_sb[:], rep_d[:])
            tblq_sb = spool.tile([P, VQ], bf16)
            nc.sync.dma_start(tblq_sb[:], tblq_d[:])
            xi_sb = spool.tile([P, NINST * NIW], i16)
            nc.sync.dma_start(xi_sb[:], xi_d[:])
            b1_sb = pool.tile([P, 1], f32)
            nc.sync.dma_start(b1_sb[:], b1_d[:])
            wm_sb = pool.tile([P, P], f32)
            nc.sync.dma_start(wm_sb[:], wm_d[:])
            wl_sb = pool.tile([P, P], f32)
            nc.sync.dma_start(wl_sb[:], wl_d[:])
            bm_sb = pool.tile([P, 1], f32)
            nc.sync.dma_start(bm_sb[:], bm_d[:])
            bl_sb = pool.tile([P, 1], f32)
            nc.sync.dma_start(bl_sb[:], bl_d[:])

            # table replication: bf16 [128, VQ] staging -> f32 [128, V] table
            tbl_sb = pool.tile([P, V], f32)
            ci = 0
            for a in range(4):
                for u0, w in tiles:
                    psr = prpool.tile([P, COPYW], f32)
                    for v0 in range(0, w, 512):
                        vw = min(512, w - v0)
                        nc.tensor.matmul(
                            psr[:, v0 : v0 + vw],
                            rep_sb[32 * a : 32 * (a + 1), :],
                            tblq_sb[32 * a : 32 * (a + 1), u0 + v0 : u0 + v0 + vw],
                            tile_position=(32 * a, 0),
                        )
                    dst = tbl_sb[:, a * VQ + u0 : a * VQ + u0 + w]
                    if ci % 2 == 0:
                        nc.scalar.activation(
                            out=dst,
                            in_=psr[:, :w],
                            func=mybir.ActivationFunctionType.Copy,
                        )
                    else:
                        nc.vector.tensor_scalar_mul(dst, psr[:, :w], 1.0)
                    ci += 1

            hall = pool.tile([P, SLOTS], f32)
            for k in range(NINST):
                gath = gpool.tile([P, NI], f32)
                nc.gpsimd.ap_gather(
                    out_ap=gath[:],
                    in_ap=tbl_sb[:],
                    idxs_ap=xi_sb[:, k * NIW : (k + 1) * NIW],
                    channels=P,
                    num_elems=V,
                    d=1,
                    num_idxs=NI,
                )
                nc.vector.tensor_reduce(
                    out=hall[:, k * RPG : (k + 1) * RPG],
                    in_=gath[:].rearrange("p (r s) -> p r s", s=S),
                    axis=mybir.AxisListType.X,
                    op=mybir.AluOpType.add,
                )

            hr = pool.tile([P, SLOTS], f32)
            nc.scalar.activation(
                out=hr[:],
                in_=hall[:],
                func=mybir.ActivationFunctionType.Relu,
                bias=b1_sb[:],
            )

            om_ps = pspool.tile([P, SLOTS], f32)
            nc.tensor.matmul(om_ps[:], wm_sb[:], hr[:])
            ol_ps = pspool.tile([P, SLOTS], f32)
            nc.tensor.matmul(ol_ps[:], wl_sb[:], hr[:])

            o_sb = pool.tile([P, 2 * SLOTS], f32)
            nc.vector.tensor_scalar(
                out=o_sb[:, :SLOTS],
                in0=om_ps[:],
                scalar1=bm_sb[:],
                scalar2=None,
                op0=mybir.AluOpType.add,
            )
            nc.vector.tensor_scalar(
                out=o_sb[:, SLOTS:],
                in0=ol_ps[:],
                scalar1=bl_sb[:],
                scalar2=None,
                op0=mybir.AluOpType.add,
            )
            nc.sync.dma_start(out_d[:], o_sb[:])

    nc.compile()
    return nc


def _get_nc(repeat=1):
    key = ("nc", repeat)
    if key not in _CACHE:
        _CACHE[key] = _build_nc(repeat)
    return _CACHE[key]


def _prep_inputs(x, enc1_w, enc1_b, mean_w, mean_b, logvar_w, logvar_b):
    import ml_dtypes

    x = np.asarray(x)
    assert x.shape == (B, S)
    # row r of core c = global row c*RPC + r; within a core, row r is
    # handled by group g = r % G at slot n = r // G; instruction k covers
    # slots k*RPG .. k*RPG+RPG-1.
    xs = x.astype(np.int16).reshape(NCORES, SLOTS, G, S)  # [c, n, g, s]
    stream = xs.transpose(0, 2, 1, 3).reshape(NCORES, G, NINST, NI)  # [c,g,k,i]
    wrapped = stream.reshape(NCORES, G, NINST, NIW, 16).transpose(0, 1, 4, 2, 3)
    xi16 = np.ascontiguousarray(wrapped.reshape(NCORES, G * 16, NINST * NIW))

    w = np.asarray(enc1_w, dtype=np.float32)  # [H, V]
    # tblq[32a+h, u] = w[h, a*VQ+u] for h<16; rows 32a+16..32a+31 unused
    tblq = np.zeros((P, VQ), dtype=ml_dtypes.bfloat16)
    wq = w.reshape(H, 4, VQ)
    for a in range(4):
        tblq[32 * a : 32 * a + H, :] = wq[:, a, :].astype(ml_dtypes.bfloat16)

    rep = np.zeros((P, P), dtype=ml_dtypes.bfloat16)
    for a in range(4):
        for m in range(P):
            rep[32 * a + (m % H), m] = 1.0

    pidx = np.arange(P) % H
    b1rep = np.asarray(enc1_b, dtype=np.float32)[pidx][:, None].copy()
    bmrep = np.asarray(mean_b, dtype=np.float32)[pidx][:, None].copy()
    blrep = np.asarray(logvar_b, dtype=np.float32)[pidx][:, None].copy()

    def blockdiag(wmat):
        wmat = np.asarray(wmat, dtype=np.float32)  # [O, H]
        bd = np.zeros((P, P), dtype=np.float32)
        for g in range(G):
            bd[g * H : (g + 1) * H, g * O : (g + 1) * O] = wmat.T
        return bd

    wmbd = blockdiag(mean_w)
    wlbd = blockdiag(logvar_w)
    return [
        {
            "tblq": tblq,
            "rep": rep,
            "xi16": xi16[c],
            "b1rep": b1rep,
            "wmbd": wmbd,
            "wlbd": wlbd,
            "bmrep": bmrep,
            "blrep": blrep,
        }
        for c in range(NCORES)
    ]


def _unscramble(out_core):
    # out_core [128, 2*SLOTS]: partition 16g+o, col n -> row n*G+g
    o = out_core.reshape(G, O, 2, SLOTS)  # [g, o, {m,l}, n]
    o = o.transpose(2, 3, 0, 1)  # [{m,l}, n, g, o]
    return o.reshape(2, SLOTS * G, O)  # rows r = n*G + g


def _run(in_maps, trace=False, repeat=1):
    from concourse.bass_utils import run_bass_kernel_spmd

    nc = _get_nc(repeat)
    core_ids = list(range(NCORES))
    res = run_bass_kernel_spmd(nc, in_maps, core_ids, trace=trace)
    mean = np.empty((B, O), dtype=np.float32)
    logvar = np.empty((B, O), dtype=np.float32)
    for c in core_ids:
        ml = _unscramble(res.results[c]["out"])
        mean[c * RPC : (c + 1) * RPC] = ml[0]
        logvar[c * RPC : (c + 1) * RPC] = ml[1]
    return mean, logvar, res


def kernel(x, enc1_w, enc1_b, mean_w, mean_b, logvar_w, logvar_b):
    in_maps = _prep_inputs(x, enc1_w, enc1_b, mean_w, mean_b, logvar_w, logvar_b)
    mean, logvar, _ = _run(in_maps, trace=False)
    return mean, logvar
